# revision 25
# baseline (speedup 1.0000x reference)
"""Channel attention kernel for Trainium2, 8-core data parallel.

Computes, per batch b:
    X   = x[b].reshape(C, H*W)            # (512, 2304)
    G   = X @ X.T                         # (512, 512) Gram
    A   = softmax(G, axis=1)
    agg = A @ X                           # (512, 2304)
    out[b] = x[b] + scale * agg

Sharding: pure data parallel over the batch dim n=64 -> 8 batches per core.

Per-core pipeline (fp8e4m3 DoubleRow matmuls = 2 MACs/cell/cycle; fp32
accumulation in PSUM; residual path exact fp32):
  1. DMA x[b] into 4 SBUF tiles xs[cb]=[128,2304] f32 (2 column pieces).
  2. PE-transposes X (f32, straight from xs so nothing gates them) ->
     fp32 PSUM; the evac copies CONVERT to fp8 zxT[j]=[128, 2*512]
     pairing d-blocks (2j, 2j+1): DoubleRow operands for mm1.
  3. fp8 casts of X for mm2's moving operand, off the critical path:
     zx[0] (c-blocks 0,1) on the idle GPSIMD, zx[1] (c-blocks 2,3) on
     ACT. Only mm2 consumes these, ~15us after the loads.
  4. mm1 upper triangle only (G symmetric): G[mb][:, mb*128:] via 9
     DoubleRow matmuls; lower blocks filled by PE-transposing the
     SBUF copies of earlier rows' off-diagonal blocks.
  5. softmax: row max (DVE, negated) -> exp with bias + fused row-sum
     (ACT accum_out, fp8 output es). Reciprocal + scale deferred:
     one batched [128,4] reciprocal (DVE) + one ACT scale-mul.
  6. PE-transpose es -> ett[mb] fp8 (fp8 transposes write PSUM at
     element step 2; strided ACT copy evacuates), one block behind mm1.
  7. mm2: Y (PSUM f32) += ett[mb]^T-pairs @ zx[jp][:,:,chunk], DoubleRow.
  8. out = (Y * (scale/rowsum)) + X via DVE scalar_tensor_tensor into a
     full-row o tile [128,2304]; one contiguous DMA out per c-block.
Emission is software-pipelined: batch b+1's loads+casts are emitted
between mm1(b) and mm2(b) so the input DMAs are not queued behind
batch b's output DMAs (sync-queue head-of-line blocking).
"""

import numpy as np
from contextlib import ExitStack

import concourse.bass as bass
import concourse.bacc as bacc
import concourse.tile as tile
from concourse import mybir
from concourse.masks import make_identity
from concourse.bass_utils import run_bass_kernel_spmd

N_CORES = 8
N, C, H, W = 64, 512, 48, 48
HW = H * W                    # 2304
B = N // N_CORES              # 8 batches per core
P = 128
NCB = C // P                  # 4 c-blocks
NDB = HW // P                 # 18 d-blocks
NDP = NDB // 2                # 9 d-pairs (DoubleRow K=256 steps)
F32 = mybir.dt.float32
F8 = mybir.dt.float8e4
DR = mybir.MatmulPerfMode.DoubleRow

CHUNKS = [(i * 512, min(512, HW - i * 512)) for i in range((HW + 511) // 512)]
PIECES = ((0, 1152), (1152, HW))
# zxT evac pairs handled by DVE (rest ACT). At a batch boundary DVE is
# still draining the previous batch's residual chunks for ~half the
# transpose phase, so ANY pair given to DVE either backpressures the PE
# via the transpose PSUM pool (early pairs) or delays mm1's last
# contraction steps (late pairs): ACT takes them all.
DVE_PAIRS = set()

AX = mybir.AxisListType.X
MULT = mybir.AluOpType.mult
ADD = mybir.AluOpType.add
EXP = mybir.ActivationFunctionType.Exp


def _build(nb=B):
    nc = bacc.Bacc()
    x_d = nc.dram_tensor("x", [nb, C, HW], F32, kind="ExternalInput")
    s_d = nc.dram_tensor("scale", [1], F32, kind="ExternalInput")
    o_d = nc.dram_tensor("out", [nb, C, HW], F32, kind="ExternalOutput")

    with tile.TileContext(nc) as tc:
        with ExitStack() as ctx:
            singles = ctx.enter_context(tc.tile_pool(name="singles", bufs=1))
            xpool = ctx.enter_context(tc.tile_pool(name="xp", bufs=8))
            zxpool = ctx.enter_context(tc.tile_pool(name="zxp", bufs=4))
            ztpool = ctx.enter_context(tc.tile_pool(name="ztp", bufs=19))
            epool = ctx.enter_context(tc.tile_pool(name="ep", bufs=5))
            gsbpool = ctx.enter_context(tc.tile_pool(name="gsb", bufs=5))
            etpool = ctx.enter_context(tc.tile_pool(name="etp", bufs=5))
            opool = ctx.enter_context(tc.tile_pool(name="op", bufs=6))
            stats = ctx.enter_context(tc.tile_pool(name="st", bufs=24))
            tpsum = ctx.enter_context(
                tc.tile_pool(name="tps", bufs=3, space="PSUM"))
            gpsum = ctx.enter_context(
                tc.tile_pool(name="gps", bufs=2, space="PSUM"))
            epsum = ctx.enter_context(
                tc.tile_pool(name="eps", bufs=1, space="PSUM"))
            ypsum = ctx.enter_context(
                tc.tile_pool(name="yps", bufs=2, space="PSUM"))

            identity = singles.tile([P, P], F32)
            make_identity(nc, identity[:])
            id8 = singles.tile([P, P], F8)
            nc.scalar.copy(out=id8[:], in_=identity[:])
            scale_sb = singles.tile([P, 1], F32)
            nc.sync.dma_start(out=scale_sb[:], in_=s_d.broadcast_to([P, 1]))

            # PE observes the gpsimd-produced f32 identity and the
            # ACT-produced fp8 identity once here; real matmuls then never
            # wait on those semaphores (matmuls have one sync-wait slot).
            warm = gpsum.tile([P, C], F32, tag="g")
            nc.tensor.transpose(warm[:, :P], identity[:], identity[:])
            w8 = epsum.tile([P, 2 * C], F8, tag="e8")
            nc.tensor.transpose(w8[:, 0:2 * P:2], id8[:], id8[:])
            # ~3.7us of dummy PE work while the first DMAs land: keeps the
            # HAM activity window busy so batch 0 runs at full clock.
            for w in range(28):
                wt = gpsum.tile([P, C], F32, tag="g", name=f"warm{w}")
                nc.tensor.transpose(wt[:, :P], identity[:], identity[:])

            state = {}

            def emit_loads(b):
                xs = [xpool.tile([P, HW], F32, tag="x", name=f"x{cb}")
                      for cb in range(NCB)]
                zx = [zxpool.tile([P, 2 * HW], F8, tag="zx", name=f"zx{j}")
                      for j in range(2)]
                for (p0, p1) in PIECES:
                    for cb in range(NCB):
                        nc.sync.dma_start(
                            out=xs[cb][:, p0:p1],
                            in_=x_d[b, cb * P:(cb + 1) * P, p0:p1])
                    # gpsimd's share of the fp8 casts (zx[0]); ACT's share
                    # (zx[1]) is emitted later inside emit_compute_head so
                    # it queues after the transpose evacuations.
                    for cb in (0, 1):
                        off = (cb % 2) * HW
                        nc.gpsimd.tensor_copy(
                            out=zx[cb // 2][:, off + p0:off + p1],
                            in_=xs[cb][:, p0:p1])
                state[b] = (xs, zx)

            def emit_compute_head(b):
                """Transposes + mm1 + softmax (+ pipelined E transpose)."""
                xs, zx = state[b]

                # f32 transposes straight from xs (nothing gates them);
                # the evac copies convert f32 PSUM -> fp8 zxT.
                zxT = [ztpool.tile([P, 2 * C], F8, tag="zt", name=f"zxT{j}")
                       for j in range(NDP)]
                for kb in range(NDB):
                    ps = tpsum.tile([P, C], F32, tag="tps")
                    for cb in range(NCB):
                        nc.tensor.transpose(
                            ps[:, cb * P:(cb + 1) * P],
                            xs[cb][:, kb * P:(kb + 1) * P],
                            identity[:])
                    half = (kb % 2) * C
                    if (kb // 2) in DVE_PAIRS:
                        nc.vector.tensor_copy(
                            out=zxT[kb // 2][:, half:half + C], in_=ps[:])
                    else:
                        nc.scalar.copy(
                            out=zxT[kb // 2][:, half:half + C], in_=ps[:])


                es = []
                etts = []
                s_tile = stats.tile([P, NCB], F32, tag="s")

                def emit_etrans(m):
                    ps = epsum.tile([P, 2 * C], F8, tag="e8")
                    for kb in range(NCB):
                        nc.tensor.transpose(
                            ps[:, kb * 2 * P:(kb + 1) * 2 * P:2],
                            es[m][:, kb * P:(kb + 1) * P], id8[:])
                    ett = etpool.tile([P, C], F8, tag="et", name=f"eTT{m}")
                    nc.scalar.copy(out=ett[:], in_=ps[:, 0:2 * C:2])
                    return ett

                # zx[1] casts as a DVE block ahead of mm1: they run
                # behind the previous batch's residual drain and finish
                # around mm1 start, well before mm2 consumes zx[1].
                # piece0 of both c-blocks first (mm2's early chunks read
                # the low columns of BOTH halves).
                for ccb, cpi in ((2, 0), (3, 0), (2, 1), (3, 1)):
                    cp0, cp1 = PIECES[cpi]
                    coff = (ccb % 2) * HW
                    nc.vector.tensor_copy(
                        out=zx[1][:, coff + cp0:coff + cp1],
                        in_=xs[ccb][:, cp0:cp1])

                for mb in range(NCB):
                    G = gpsum.tile([P, C], F32, tag="g", name=f"G{mb}")
                    # mm1 is LDWEIGHTS-bound (36 256-col loads) so the
                    # full-width moving operand costs nothing extra
                    for j in range(NDP):
                        v = zxT[j][:].rearrange("p (k m) -> p k m", k=2)
                        nc.tensor.matmul(
                            G[:],
                            v[:, :, mb * P:(mb + 1) * P],
                            v,
                            start=(j == 0), stop=(j == NDP - 1),
                            perf_mode=DR)
                    # Evacuate G to SBUF immediately (ACT): the PSUM bank
                    # then has a single prompt reader, so the next-next
                    # mm1 group's start matmul never waits on the
                    # rmax->exp chain (which trails on busy DVE/ACT).
                    gs = gsbpool.tile([P, C], F32, tag="gs")
                    nc.scalar.copy(out=gs[:], in_=G[:])
                    neg_m = stats.tile([P, 1], F32, tag="negm")
                    nc.vector.reduce_max(
                        out=neg_m[:], in_=gs[:], axis=AX, negate=True)
                    e = epool.tile([P, C], F8, tag="e")
                    nc.scalar.activation(
                        out=e[:], in_=gs[:], func=EXP,
                        bias=neg_m[:], scale=1.0,
                        accum_out=s_tile[:, mb:mb + 1])
                    es.append(e)
                    if mb >= 1:
                        etts.append(emit_etrans(mb - 1))
                etts.append(emit_etrans(NCB - 1))

                # batched normalization: one reciprocal + one ACT scale-mul
                rs = stats.tile([P, NCB], F32, tag="rs")
                nc.vector.reciprocal(out=rs[:], in_=s_tile[:])
                alpha = stats.tile([P, NCB], F32, tag="al")
                nc.scalar.mul(alpha[:], rs[:], scale_sb[:])
                state[b] = (xs, zx, etts, alpha)

            def emit_compute_tail(b):
                """mm2 + fused residual + store."""
                xs, zx, etts, alpha = state.pop(b)
                for mb in range(NCB):
                    ev = etts[mb][:].rearrange("p (k m) -> p k m", k=NCB)
                    o = opool.tile([P, HW], F32, tag="o")
                    for (c0, csz) in CHUNKS:
                        y = ypsum.tile([P, 512], F32, tag="y")
                        for jp in range(2):
                            zv = zx[jp][:].rearrange(
                                "p (k d) -> p k d", k=2)
                            nc.tensor.matmul(
                                y[:, :csz],
                                ev[:, 2 * jp:2 * jp + 2, :],
                                zv[:, :, c0:c0 + csz],
                                start=(jp == 0), stop=(jp == 1),
                                perf_mode=DR)
                        nc.vector.scalar_tensor_tensor(
                            out=o[:, c0:c0 + csz], in0=y[:, :csz],
                            scalar=alpha[:, mb:mb + 1],
                            in1=xs[mb][:, c0:c0 + csz],
                            op0=MULT, op1=ADD)
                    nc.sync.dma_start(
                        out=o_d[b, mb * P:(mb + 1) * P, :], in_=o[:])

            emit_loads(0)
            for b in range(nb):
                emit_compute_head(b)
                if b + 1 < nb:
                    emit_loads(b + 1)
                emit_compute_tail(b)
    nc.finalize()
    return nc


def _ensure_ntff_hook():
    """Install the axon NTFF profiling hook if the image's antenv lacks it.

    Only needed for trace=True runs (local perf iteration); the grading
    path never calls this.
    """
    import sys
    import types
    try:
        from antenv import axon_hooks  # noqa: F401
        return
    except ImportError:
        pass
    mod = types.ModuleType("antenv.axon_hooks")
    _h = {"hook": None}
    mod.set_axon_ntff_profile_hook = lambda h: _h.__setitem__("hook", h)
    mod.get_axon_ntff_profile_hook = lambda: _h["hook"]
    sys.modules["antenv.axon_hooks"] = mod
    import antenv
    antenv.axon_hooks = mod
    try:
        from trn_agent_boot.trn_boot import _ntff_profile_via_ctypes
        hook = _ntff_profile_via_ctypes("/opt/axon/libaxon_pjrt.so")
        if hook is not None:
            mod.set_axon_ntff_profile_hook(hook)
    except Exception:
        pass


_NC_CACHE = {}


def _get_nc():
    if "nc" not in _NC_CACHE:
        _NC_CACHE["nc"] = _build()
    return _NC_CACHE["nc"]


def kernel(x, scale, trace=False, use_f32r=True):
    x = np.ascontiguousarray(x, dtype=np.float32)
    scale = np.ascontiguousarray(scale, dtype=np.float32)
    if trace:
        _ensure_ntff_hook()
    nc = _get_nc()
    xr = x.reshape(N, C, HW)
    in_maps = [
        {"x": xr[i * B:(i + 1) * B], "scale": scale}
        for i in range(N_CORES)
    ]
    res = run_bass_kernel_spmd(
        nc, in_maps, core_ids=list(range(N_CORES)), trace=trace)
    out = np.concatenate([r["out"] for r in res.results], axis=0)
    out = out.reshape(N, C, H, W)
    if trace:
        kernel.last_exec_time_ns = res.exec_time_ns
        kernel.last_results = res
    return out


# revision 28
# speedup vs baseline: 1.0646x; 1.0646x over previous
"""Channel attention kernel for Trainium2, 8-core data parallel.

Computes, per batch b:
    X   = x[b].reshape(C, H*W)            # (512, 2304)
    G   = X @ X.T                         # (512, 512) Gram
    A   = softmax(G, axis=1)
    agg = A @ X                           # (512, 2304)
    out[b] = x[b] + scale * agg

Sharding: pure data parallel over the batch dim n=64 -> 8 batches per core.

Per-core pipeline (fp8e4m3 DoubleRow matmuls = 2 MACs/cell/cycle; fp32
accumulation in PSUM; residual path exact fp32):
  1. DMA x[b] into 4 SBUF tiles xs[cb]=[128,2304] f32 (2 column pieces).
  2. PE-transposes X (f32, straight from xs so nothing gates them) ->
     fp32 PSUM; the evac copies CONVERT to fp8 zxT[j]=[128, 2*512]
     pairing d-blocks (2j, 2j+1): DoubleRow operands for mm1.
  3. fp8 casts of X for mm2's moving operand, off the critical path:
     zx[0] (c-blocks 0,1) on the idle GPSIMD, zx[1] (c-blocks 2,3) on
     ACT. Only mm2 consumes these, ~15us after the loads.
  4. mm1 upper triangle only (G symmetric): G[mb][:, mb*128:] via 9
     DoubleRow matmuls; lower blocks filled by PE-transposing the
     SBUF copies of earlier rows' off-diagonal blocks.
  5. softmax: row max (DVE, negated) -> exp with bias + fused row-sum
     (ACT accum_out, fp8 output es). Reciprocal + scale deferred:
     one batched [128,4] reciprocal (DVE) + one ACT scale-mul.
  6. PE-transpose es -> ett[mb] fp8 (fp8 transposes write PSUM at
     element step 2; strided ACT copy evacuates), one block behind mm1.
  7. mm2: Y (PSUM f32) += ett[mb]^T-pairs @ zx[jp][:,:,chunk], DoubleRow.
  8. out = (Y * (scale/rowsum)) + X via DVE scalar_tensor_tensor into a
     full-row o tile [128,2304]; one contiguous DMA out per c-block.
Emission is software-pipelined: batch b+1's loads+casts are emitted
between mm1(b) and mm2(b) so the input DMAs are not queued behind
batch b's output DMAs (sync-queue head-of-line blocking).
"""

import numpy as np
from contextlib import ExitStack

import concourse.bass as bass
import concourse.bacc as bacc
import concourse.tile as tile
from concourse import mybir
from concourse.masks import make_identity
from concourse.bass_utils import run_bass_kernel_spmd

N_CORES = 8
N, C, H, W = 64, 512, 48, 48
HW = H * W                    # 2304
B = N // N_CORES              # 8 batches per core
P = 128
NCB = C // P                  # 4 c-blocks
NDB = HW // P                 # 18 d-blocks
NDP = NDB // 2                # 9 d-pairs (DoubleRow K=256 steps)
F32 = mybir.dt.float32
F8 = mybir.dt.float8e4
DR = mybir.MatmulPerfMode.DoubleRow

CHUNKS = [(i * 512, min(512, HW - i * 512)) for i in range((HW + 511) // 512)]
PIECES = ((0, 1152), (1152, HW))
# zxT evac pairs handled by DVE (rest ACT). Late pairs go to DVE: at a
# batch boundary DVE is still draining the previous batch's residual
# chunks, so early-kb evacuations would backpressure the PE via the
# transpose PSUM pool.
DVE_PAIRS = {6, 7, 8}

AX = mybir.AxisListType.X
MULT = mybir.AluOpType.mult
ADD = mybir.AluOpType.add
EXP = mybir.ActivationFunctionType.Exp


def _build(nb=B):
    nc = bacc.Bacc()
    x_d = nc.dram_tensor("x", [nb, C, HW], F32, kind="ExternalInput")
    s_d = nc.dram_tensor("scale", [1], F32, kind="ExternalInput")
    o_d = nc.dram_tensor("out", [nb, C, HW], F32, kind="ExternalOutput")

    with tile.TileContext(nc) as tc:
        with ExitStack() as ctx:
            singles = ctx.enter_context(tc.tile_pool(name="singles", bufs=1))
            xpool = ctx.enter_context(tc.tile_pool(name="xp", bufs=8))
            zxpool = ctx.enter_context(tc.tile_pool(name="zxp", bufs=4))
            ztpool = ctx.enter_context(tc.tile_pool(name="ztp", bufs=19))
            epool = ctx.enter_context(tc.tile_pool(name="ep", bufs=5))
            etpool = ctx.enter_context(tc.tile_pool(name="etp", bufs=5))
            opool = ctx.enter_context(tc.tile_pool(name="op", bufs=6))
            stats = ctx.enter_context(tc.tile_pool(name="st", bufs=24))
            tpsum = ctx.enter_context(
                tc.tile_pool(name="tps", bufs=3, space="PSUM"))
            gpsum = ctx.enter_context(
                tc.tile_pool(name="gps", bufs=2, space="PSUM"))
            epsum = ctx.enter_context(
                tc.tile_pool(name="eps", bufs=1, space="PSUM"))
            ypsum = ctx.enter_context(
                tc.tile_pool(name="yps", bufs=2, space="PSUM"))

            identity = singles.tile([P, P], F32)
            make_identity(nc, identity[:])
            id8 = singles.tile([P, P], F8)
            nc.scalar.copy(out=id8[:], in_=identity[:])
            scale_sb = singles.tile([P, 1], F32)
            nc.sync.dma_start(out=scale_sb[:], in_=s_d.broadcast_to([P, 1]))

            # PE observes the gpsimd-produced f32 identity and the
            # ACT-produced fp8 identity once here; real matmuls then never
            # wait on those semaphores (matmuls have one sync-wait slot).
            warm = gpsum.tile([P, C], F32, tag="g")
            nc.tensor.transpose(warm[:, :P], identity[:], identity[:])
            w8 = epsum.tile([P, 2 * C], F8, tag="e8")
            nc.tensor.transpose(w8[:, 0:2 * P:2], id8[:], id8[:])
            # ~3.7us of dummy PE work while the first DMAs land: keeps the
            # HAM activity window busy so batch 0 runs at full clock.
            for w in range(28):
                wt = gpsum.tile([P, C], F32, tag="g", name=f"warm{w}")
                nc.tensor.transpose(wt[:, :P], identity[:], identity[:])

            state = {}

            def emit_loads(b):
                xs = [xpool.tile([P, HW], F32, tag="x", name=f"x{cb}")
                      for cb in range(NCB)]
                zx = [zxpool.tile([P, 2 * HW], F8, tag="zx", name=f"zx{j}")
                      for j in range(2)]
                for (p0, p1) in PIECES:
                    for cb in range(NCB):
                        nc.sync.dma_start(
                            out=xs[cb][:, p0:p1],
                            in_=x_d[b, cb * P:(cb + 1) * P, p0:p1])
                    # gpsimd's share of the fp8 casts (zx[0]); ACT's share
                    # (zx[1]) is emitted later inside emit_compute_head so
                    # it queues after the transpose evacuations.
                    for cb in (0, 1):
                        off = (cb % 2) * HW
                        nc.gpsimd.tensor_copy(
                            out=zx[cb // 2][:, off + p0:off + p1],
                            in_=xs[cb][:, p0:p1])
                state[b] = (xs, zx)

            def emit_compute_head(b):
                """Transposes + mm1 + softmax (+ pipelined E transpose)."""
                xs, zx = state[b]

                # f32 transposes straight from xs (nothing gates them);
                # the evac copies convert f32 PSUM -> fp8 zxT.
                zxT = [ztpool.tile([P, 2 * C], F8, tag="zt", name=f"zxT{j}")
                       for j in range(NDP)]
                for kb in range(NDB):
                    ps = tpsum.tile([P, C], F32, tag="tps")
                    for cb in range(NCB):
                        nc.tensor.transpose(
                            ps[:, cb * P:(cb + 1) * P],
                            xs[cb][:, kb * P:(kb + 1) * P],
                            identity[:])
                    half = (kb % 2) * C
                    if (kb // 2) in DVE_PAIRS:
                        nc.vector.tensor_copy(
                            out=zxT[kb // 2][:, half:half + C], in_=ps[:])
                    else:
                        nc.scalar.copy(
                            out=zxT[kb // 2][:, half:half + C], in_=ps[:])


                es = []
                etts = []
                s_tile = stats.tile([P, NCB], F32, tag="s")

                def emit_etrans(m):
                    ps = epsum.tile([P, 2 * C], F8, tag="e8")
                    for kb in range(NCB):
                        nc.tensor.transpose(
                            ps[:, kb * 2 * P:(kb + 1) * 2 * P:2],
                            es[m][:, kb * P:(kb + 1) * P], id8[:])
                    ett = etpool.tile([P, C], F8, tag="et", name=f"eTT{m}")
                    nc.scalar.copy(out=ett[:], in_=ps[:, 0:2 * C:2])
                    return ett

                # ACT's share of the fp8 casts, emitted AFTER the evac
                # copies in the ACT FIFO: during the transpose phase ACT
                # must not be busy casting or the PE stalls on the
                # transpose PSUM pool; mm2 needs zx only much later.
                for (p0, p1) in PIECES:
                    for cb in (2, 3):
                        off = (cb % 2) * HW
                        nc.scalar.copy(
                            out=zx[cb // 2][:, off + p0:off + p1],
                            in_=xs[cb][:, p0:p1])

                for mb in range(NCB):
                    # G banks: groups 0,1 from gpsum, groups 2,3 borrow
                    # the y pool (idle during mm1). This way group 2's
                    # start matmul waits on a long-finished residual
                    # chunk instead of on group 0's rmax->exp chain,
                    # which trails on the busy DVE/ACT queues.
                    pool = gpsum if mb < 2 else ypsum
                    tag = "g" if mb < 2 else "y"
                    G = pool.tile([P, C], F32, tag=tag, name=f"G{mb}")
                    # mm1 is LDWEIGHTS-bound (36 256-col loads) so the
                    # full-width moving operand costs nothing extra
                    for j in range(NDP):
                        v = zxT[j][:].rearrange("p (k m) -> p k m", k=2)
                        nc.tensor.matmul(
                            G[:],
                            v[:, :, mb * P:(mb + 1) * P],
                            v,
                            start=(j == 0), stop=(j == NDP - 1),
                            perf_mode=DR)
                    neg_m = stats.tile([P, 1], F32, tag="negm")
                    nc.vector.reduce_max(
                        out=neg_m[:], in_=G[:], axis=AX, negate=True)
                    e = epool.tile([P, C], F8, tag="e")
                    nc.scalar.activation(
                        out=e[:], in_=G[:], func=EXP,
                        bias=neg_m[:], scale=1.0,
                        accum_out=s_tile[:, mb:mb + 1])
                    es.append(e)
                    if mb >= 1:
                        etts.append(emit_etrans(mb - 1))
                etts.append(emit_etrans(NCB - 1))

                # batched normalization: one reciprocal + one ACT scale-mul
                rs = stats.tile([P, NCB], F32, tag="rs")
                nc.vector.reciprocal(out=rs[:], in_=s_tile[:])
                alpha = stats.tile([P, NCB], F32, tag="al")
                nc.scalar.mul(alpha[:], rs[:], scale_sb[:])
                state[b] = (xs, zx, etts, alpha)

            def emit_compute_tail(b):
                """mm2 + fused residual + store."""
                xs, zx, etts, alpha = state.pop(b)
                for mb in range(NCB):
                    ev = etts[mb][:].rearrange("p (k m) -> p k m", k=NCB)
                    o = opool.tile([P, HW], F32, tag="o")
                    for (c0, csz) in CHUNKS:
                        y = ypsum.tile([P, 512], F32, tag="y")
                        for jp in range(2):
                            zv = zx[jp][:].rearrange(
                                "p (k d) -> p k d", k=2)
                            nc.tensor.matmul(
                                y[:, :csz],
                                ev[:, 2 * jp:2 * jp + 2, :],
                                zv[:, :, c0:c0 + csz],
                                start=(jp == 0), stop=(jp == 1),
                                perf_mode=DR)
                        nc.vector.scalar_tensor_tensor(
                            out=o[:, c0:c0 + csz], in0=y[:, :csz],
                            scalar=alpha[:, mb:mb + 1],
                            in1=xs[mb][:, c0:c0 + csz],
                            op0=MULT, op1=ADD)
                    nc.sync.dma_start(
                        out=o_d[b, mb * P:(mb + 1) * P, :], in_=o[:])

            emit_loads(0)
            for b in range(nb):
                emit_compute_head(b)
                if b + 1 < nb:
                    emit_loads(b + 1)
                emit_compute_tail(b)
    nc.finalize()
    return nc


def _ensure_ntff_hook():
    """Install the axon NTFF profiling hook if the image's antenv lacks it.

    Only needed for trace=True runs (local perf iteration); the grading
    path never calls this.
    """
    import sys
    import types
    try:
        from antenv import axon_hooks  # noqa: F401
        return
    except ImportError:
        pass
    mod = types.ModuleType("antenv.axon_hooks")
    _h = {"hook": None}
    mod.set_axon_ntff_profile_hook = lambda h: _h.__setitem__("hook", h)
    mod.get_axon_ntff_profile_hook = lambda: _h["hook"]
    sys.modules["antenv.axon_hooks"] = mod
    import antenv
    antenv.axon_hooks = mod
    try:
        from trn_agent_boot.trn_boot import _ntff_profile_via_ctypes
        hook = _ntff_profile_via_ctypes("/opt/axon/libaxon_pjrt.so")
        if hook is not None:
            mod.set_axon_ntff_profile_hook(hook)
    except Exception:
        pass


_NC_CACHE = {}


def _get_nc():
    if "nc" not in _NC_CACHE:
        _NC_CACHE["nc"] = _build()
    return _NC_CACHE["nc"]


def kernel(x, scale, trace=False, use_f32r=True):
    x = np.ascontiguousarray(x, dtype=np.float32)
    scale = np.ascontiguousarray(scale, dtype=np.float32)
    if trace:
        _ensure_ntff_hook()
    nc = _get_nc()
    xr = x.reshape(N, C, HW)
    in_maps = [
        {"x": xr[i * B:(i + 1) * B], "scale": scale}
        for i in range(N_CORES)
    ]
    res = run_bass_kernel_spmd(
        nc, in_maps, core_ids=list(range(N_CORES)), trace=trace)
    out = np.concatenate([r["out"] for r in res.results], axis=0)
    out = out.reshape(N, C, H, W)
    if trace:
        kernel.last_exec_time_ns = res.exec_time_ns
        kernel.last_results = res
    return out
